# revision 9
# baseline (speedup 1.0000x reference)
"""Block-sparse linear y = x @ W^T on 8 Trainium2 NeuronCores.

The 32x32 block structure (50% block density, random scatter) is not
exploitable on the 128x128 PE array (matmul cost is moving-rows only, and
neither 128-aligned skip opportunities nor packable col-quadruples occur at
this density), so W^T is densified on the host (cheap scatter-add) and run
as a dense GEMM sharded 4-way over tokens x 2-way over out_features
(8 cores, no collectives).

All operands are bf16: the PE runs bf16 at the same 1 cycle/row as
float32r, but DMA traffic halves (x 4MB + W 4MB + y 2MB per core), which
puts DMA (~28us) far under the PE stream (~59us = 256 matmuls x 512 rows
@ ~2.22GHz) and makes the schedule trivially overlappable. bf16 rounding
of inputs + outputs keeps rel err ~1e-3, well inside the 2e-2 gate.

Schedule per core, two phases with no psum partials:
 (1) out-half n=0, k-outer m-inner: x^T k-tiles and W tiles stream in
     small-first chunks; 8 psum banks accumulate the full K; drains
     (DVE fp32->bf16 copies + 2 batched y DMAs) overlap phase 2.
 (2) out-half n=1, m-outer k-inner: each bank runs its full 16-k chain
     then drains immediately, so only the last bank's copy + 128KB DMA
     sit after the final matmul.
A dozen tiny (64-row) warmup matmuls ramp the PE clock gate during the
DMA head so real matmuls run at full clock almost immediately.

dma_start count is kept to ~19 (vs ~50 before): the end-of-kernel barrier
lowers to one EVENT_SEMAPHORE instruction per wait per engine (~90ns each,
serial), so the epilogue scales with the number of DMA semaphores.
"""

import numpy as np
import ml_dtypes

TOKENS, IN_F, OUT_F = 4096, 2048, 2048
BLOCK = 32
N_CORES = 8
TG, OG = 4, 2  # token groups x out-feature groups
T_SH = TOKENS // TG  # 1024 tokens per core
O_SH = OUT_F // OG  # 1024 out features per core
P = 128
NFREE = 512  # PSUM bank free dim (fp32)
KT = IN_F // P  # 16 k tiles
MT = T_SH // P  # 8 psum banks

N_WARM = 14  # dummy 256-row matmuls to ramp the PE clock during the DMA head

MM_DTYPE = "bfloat16"  # informational; kernel is bf16-only
TRACE = False  # set by test.py to capture an NTFF profile

_nc_cache = {}
_last_result = None  # BassKernelResults of the most recent run (for test.py)


def _build_nc():
    import concourse.mybir as mybir
    import concourse.tile as tile
    from concourse import bacc

    if "nc" in _nc_cache:
        return _nc_cache["nc"]

    bf16 = mybir.dt.bfloat16
    f32 = mybir.dt.float32

    nc = bacc.Bacc(None, target_bir_lowering=False)
    # Host-pre-blocked inputs (exact SBUF layouts; all DMAs are linear):
    # xt: x^T partition-major, [P][KT][T_SH]  (xt[p,k,t] = x^T[k*128+p, t])
    # w:  W^T per out-half,    [2][P][KT][NFREE]
    # y:  [2][P][MT][NFREE] bf16; host reassembles tokens/outs.
    xt_d = nc.dram_tensor("xt", [P, KT, T_SH], bf16, kind="ExternalInput")
    w_d = nc.dram_tensor("w", [2, P, KT, NFREE], bf16, kind="ExternalInput")
    y_d = nc.dram_tensor("y", [2, P, MT, NFREE], bf16, kind="ExternalOutput")

    with tile.TileContext(nc) as tc:
        with (
            tc.tile_pool(name="xp", bufs=1) as xp,
            tc.tile_pool(name="wp", bufs=1) as wp,
            tc.tile_pool(name="op", bufs=1) as op,
            tc.tile_pool(name="ps", bufs=1, space="PSUM") as ps,
        ):
            # Warm the PE clock gate with dummy matmuls while the first DMA
            # chunks land (~2us): PE busy-time ramps the HAM clock so real
            # matmuls run at full rate almost immediately.
            zt = xp.tile([P, 256], bf16, tag="warm", name="warm")
            nc.gpsimd.memset(zt[:], 0.0)
            warm_ps = ps.tile([P, NFREE], f32, tag="ps0", name="warm_ps")
            for _ in range(N_WARM):
                nc.tensor.matmul(
                    warm_ps[:, 0:256], zt[:, 0:P], zt[:], start=True, stop=True
                )

            xt = xp.tile([P, KT, T_SH], bf16, tag="xt", name="xt")
            wt = [
                wp.tile([P, KT, NFREE], bf16, tag=f"w{n}", name=f"w{n}")
                for n in range(2)
            ]
            ot = [
                op.tile([P, MT, NFREE], bf16, tag=f"ot{n}", name=f"ot{n}")
                for n in range(2)
            ]

            def dma_x(eng, k0, k1):
                eng.dma_start(xt[:, k0:k1, :], xt_d[:, k0:k1, :])

            def dma_w(eng, n, k0, k1):
                eng.dma_start(wt[n][:, k0:k1, :], w_d[n, :, k0:k1, :])

            # Each dma_start costs ~650ns of descriptor generation
            # (DIRECT2D) serially on its issuing sequencer, so spread the
            # issues across the three DMA-capable sequencers (sync, scalar,
            # gpsimd). The DMA rings drain roughly in issue order, so zip
            # the chunks in strict need-order round-robin across the
            # queues: a late-need chunk issued early would jump ahead in
            # the ring FIFO and starve the PE of earlier-needed data.
            dma_x(nc.sync, 0, 1)      # slot 0
            dma_w(nc.scalar, 0, 0, 1)
            dma_x(nc.gpsimd, 1, 2)
            dma_w(nc.sync, 0, 1, 2)   # slot 1
            dma_x(nc.scalar, 2, 4)
            dma_w(nc.gpsimd, 0, 2, 4)
            dma_x(nc.sync, 4, 6)      # slot 2
            dma_w(nc.scalar, 0, 4, 6)
            dma_x(nc.gpsimd, 6, 8)
            dma_w(nc.sync, 0, 6, 8)   # slot 3
            dma_x(nc.scalar, 8, 12)
            dma_w(nc.gpsimd, 0, 8, 12)
            dma_x(nc.sync, 12, 16)    # slot 4
            dma_w(nc.scalar, 0, 12, 16)
            dma_w(nc.gpsimd, 1, 0, 8)
            dma_w(nc.sync, 1, 8, 16)  # slot 5

            psums = [
                ps.tile([P, NFREE], f32, tag=f"ps{m}", name=f"ps{m}")
                for m in range(MT)
            ]

            # ---- Phase 1: n=0, k-outer m-inner (matches the DMA stream);
            # full-K accumulation in 8 psum banks, no partials. ----
            for k in range(KT):
                for m in range(MT):
                    nc.tensor.matmul(
                        psums[m][:],
                        xt[:, k, m * P : (m + 1) * P],
                        wt[0][:, k, :],
                        start=(k == 0),
                        stop=(k == KT - 1),
                    )
            for m in range(MT):
                nc.vector.tensor_copy(ot[0][:, m, :], psums[m][:])
            nc.scalar.dma_start(y_d[0, :, 0:4, :], ot[0][:, 0:4, :])
            nc.scalar.dma_start(y_d[0, :, 4:8, :], ot[0][:, 4:8, :])

            # ---- Phase 2: n=1, m-outer k-inner so each bank drains the
            # moment its chain finishes. Bank 7 runs as two 256-col
            # half-chains so only a half-bank cast + 64KB DMA trail the
            # final matmul. ----
            for m in range(MT):
                if m < MT - 1:
                    for k in range(KT):
                        nc.tensor.matmul(
                            psums[m][:],
                            xt[:, k, m * P : (m + 1) * P],
                            wt[1][:, k, :],
                            start=(k == 0),
                            stop=(k == KT - 1),
                        )
                    nc.vector.tensor_copy(ot[1][:, m, :], psums[m][:])
                else:
                    # Two 256-col half-chains in DIFFERENT psum banks (a
                    # shared bank would serialize chain B's start=True
                    # against chain A's cast). Casts split across DVE +
                    # Act, and each y DMA split into partition halves on
                    # two queues: the transfer is descriptor-count-bound
                    # (~128 partition descriptors ~= 1.2us), so two
                    # 64-partition DMAs in parallel halve the tail.
                    for h, pbank in ((0, psums[m]), (1, psums[0])):
                        c0, c1 = h * 256, (h + 1) * 256
                        for k in range(KT):
                            nc.tensor.matmul(
                                pbank[:, 0:256],
                                xt[:, k, m * P : (m + 1) * P],
                                wt[1][:, k, c0:c1],
                                start=(k == 0),
                                stop=(k == KT - 1),
                            )
                        nc.vector.tensor_copy(
                            ot[1][:, m, c0:c1], pbank[:, 0:256]
                        )
                        nc.sync.dma_start(
                            y_d[1, :, m, c0:c1], ot[1][:, m, c0:c1]
                        )
                if m == 3:
                    nc.scalar.dma_start(y_d[1, :, 0:4, :], ot[1][:, 0:4, :])
                elif m == 6:
                    nc.scalar.dma_start(y_d[1, :, 4:7, :], ot[1][:, 4:7, :])

    nc.compile()
    _nc_cache["nc"] = nc
    return nc


def _densify_wT(weight_blocks, block_rows, block_cols):
    """Scatter-add the 32x32 blocks into dense W^T [in_features, out_features]."""
    nc_blk = IN_F // BLOCK
    nr_blk = OUT_F // BLOCK
    wcr = np.zeros((nc_blk, nr_blk, BLOCK, BLOCK), np.float32)
    # block b occupies W[32r:32r+32, 32c:32c+32]; W^T gets the transposed block
    np.add.at(
        wcr,
        (block_cols.astype(np.int64), block_rows.astype(np.int64)),
        np.swapaxes(weight_blocks.astype(np.float32, copy=False), 1, 2),
    )
    return np.ascontiguousarray(wcr.transpose(0, 2, 1, 3).reshape(IN_F, OUT_F))


def _pack_core_inputs(xT_sh, wT_sh):
    """Block one core's x^T and W^T shards into the kernel's DMA layouts."""
    bf = ml_dtypes.bfloat16
    # xt [P, KT, T_SH]: xt[p,k,t] = x^T[k*128+p, t]
    xt = np.ascontiguousarray(
        xT_sh.reshape(KT, P, T_SH).transpose(1, 0, 2).astype(bf)
    )
    # w [2, P, KT, NFREE]: w[n,p,k,f] = W^T[k*128+p, n*512+f]
    w = np.ascontiguousarray(
        wT_sh.reshape(KT, P, 2, NFREE).transpose(2, 1, 0, 3).astype(bf)
    )
    return {"xt": xt, "w": w}


def kernel(x, weight_blocks, block_rows, block_cols):
    global _last_result
    from concourse.bass_utils import run_bass_kernel_spmd

    x = np.asarray(x, dtype=np.float32)
    wT = _densify_wT(
        np.asarray(weight_blocks), np.asarray(block_rows), np.asarray(block_cols)
    )
    xT = np.ascontiguousarray(x.T)

    in_maps = []
    for c in range(N_CORES):
        tg, og = divmod(c, OG)
        in_maps.append(
            _pack_core_inputs(
                xT[:, tg * T_SH : (tg + 1) * T_SH],
                wT[:, og * O_SH : (og + 1) * O_SH],
            )
        )

    nc = _build_nc()
    res = None
    for attempt in range(3):  # transient NRT device errors happen; retry
        try:
            res = run_bass_kernel_spmd(
                nc, in_maps, core_ids=list(range(N_CORES)), trace=TRACE
            )
            break
        except Exception:
            if attempt == 2:
                raise
            import time

            time.sleep(3)
    _last_result = res

    y = np.empty((TOKENS, OUT_F), np.float32)
    for c in range(N_CORES):
        tg, og = divmod(c, OG)
        # y_d [2, P, MT, NFREE] -> [m, p] tokens x [n, f] outs
        yc = (
            np.asarray(res.results[c]["y"])
            .astype(np.float32)
            .transpose(2, 1, 0, 3)
            .reshape(T_SH, O_SH)
        )
        y[tg * T_SH : (tg + 1) * T_SH, og * O_SH : (og + 1) * O_SH] = yc
    return y
